# revision 3
# baseline (speedup 1.0000x reference)
"""Trainium2 Bass kernel for nn_NeuralODE (RK3 neural ODE, 4-layer softplus MLP).

Strategy (data-parallel over batch, 8 cores):
  - Shard y0 along batch: 4096 -> 8 x 512. Replicate MLP weights.
  - Feature-major layout on device: activations are [features, batch] so every
    layer is out = W_T.T @ h with zero transposes on device. All transposes
    (y0, weights, ys output) happen on the host in numpy.
  - softplus(x) = Ln(Exp(x + b) + 1) on ScalarE; Exp runs PSUM in-place with
    the layer bias fused via the per-partition bias operand; both functions
    live in the `natural_log_exp_and_others` table set (no table switches).
  - ScalarE is the bottleneck (12 table ops per MLP eval, ~1 col/cycle,
    dtype-independent). The batch is split into TWO INDEPENDENT STREAMS
    (columns 0:256 / 256:512) so the scheduler can fill ScalarE with one
    stream's ready work while the other crosses a layer/eval boundary.
  - PHASE PIN: symmetric streams phase-lock and stall together at eval
    boundaries (~2us/eval idle). Stream B's boundary affines are gated on a
    data dependency (a zero-cost (h*0+0)+bias affine producing B's bias
    operand) against stream A's NEXT-eval L0 ln output, so B permanently
    lags A by a few act-ops and each stream's act backlog covers the
    other's serial boundary chain. Timing-only skews collapse (the
    ready-heap scheduler is work-conserving); only a real dep holds.
  - PSUM accumulation groups are BANK-scoped: interleaved groups in one bank
    corrupt each other, so every K-accumulated (m, stream) output owns a
    psum tile: L1/L2 use [128,256] tiles (4 live), L3 per-stream [64,256]
    kp tiles (2), L0 (atomic start&stop matmuls) shares a [128,512] pair
    (2). Bank budget: 8.
  - Weights/activations in fp16 (10 mantissa bits): the RK map amplifies
    per-step error ~300x over 256 steps, ruling out bf16 (final err ~0.5);
    fp16 lands ~3e-4, inside the 2e-2 gate. fp16 halves LDWEIGHTS vs fp32r
    so weight loads hide under N=256 matmuls. State and PSUM stay fp32.
  - RK combines on VectorE via the fused AFFINE_THEN_ADD custom op:
    out = (k * c + c*b3) + prev, per stream.
  - Per-step y is DMAed to a [T, 64, 512] output; host transposes to [T,B,D].
"""

import os
import sys

for _p in ("/opt/trn_rl_repo",):
    if _p not in sys.path and os.path.isdir(_p):
        sys.path.insert(0, _p)

import numpy as np

import concourse.bass as bass
import concourse.tile as tile
from concourse import bacc, mybir
from concourse.bass_utils import run_bass_kernel_spmd

F32 = mybir.dt.float32
F16 = mybir.dt.float16
AF = mybir.ActivationFunctionType


class _Bacc(bacc.Bacc):
    # Bacc whose act-table-set picker is pinned to the combined exp+ln set.
    # The default greedy picker alternates exp->exp_and_others and
    # ln->natural_log, inserting a ~2.7us ACT_TABLE_LOAD between nearly
    # every Exp/Ln pair. Blank exp/ln out of every other set (positions
    # preserved so act_func_set_id indices stay valid) so both land in
    # natural_log_exp_and_others and the load happens once.
    def insert_act_table_loads(self):
        import bass_rust as _bass_rust
        from concourse.hw_specs import get_activation_tables

        has_activation = any(
            isinstance(i, mybir.InstActivation)
            for b in self.main_func.blocks
            for i in b.instructions
        )
        if not has_activation:
            return
        tables = [
            (name, fns if name == "natural_log_exp_and_others"
             else {f for f in fns if f not in (AF.Exp, AF.Ln)})
            for name, fns in get_activation_tables(self.m.arch).items()
        ]
        _bass_rust.insert_act_table_loads(self, tables)


T = 256
B = 4096
D = 64
W = 256
N_CORES = 8
BC = B // N_CORES  # 512 batch per core
H = BC // 2        # 256-wide stream
STREAMS = (slice(0, H), slice(H, BC))
NEV = 3 * T
# per RK phase: (x scale, x bias col) and (u scale col index is bias col)
PHASE_XCOL = (0, 1, 4)   # x2 / x3 / y_r bias columns in bv
PHASE_UCOL = (2, 3, 4)   # u1 / u2 / y_new bias columns in bv


def _build_program(dt: np.float32) -> bass.Bass:
    """Build the per-core Bass program (SPMD: same program, different data)."""
    c1 = float(np.float32(np.float32(2.0 / 9.0) * dt))
    c2 = float(np.float32(np.float32(1.0 / 3.0) * dt))
    c3 = float(np.float32(np.float32(4.0 / 9.0) * dt))
    PHASE_XSCALE = (0.5, 0.75, c3)
    PHASE_USCALE = (c1, c2, c3)

    nc = _Bacc()

    # DRAM I/O (per core). All host-side pre-transposed.
    y0T_d = nc.declare_dram_parameter("y0T", [D, BC], F32, isOutput=False)
    w0T_d = nc.declare_dram_parameter("w0T", [D, W], F16, isOutput=False)
    w1T_d = nc.declare_dram_parameter("w1T", [W, W], F16, isOutput=False)
    w2T_d = nc.declare_dram_parameter("w2T", [W, W], F16, isOutput=False)
    w3T_d = nc.declare_dram_parameter("w3T", [W, D], F16, isOutput=False)
    # Hidden-layer biases packed [128, 6]: column 2l+m = b_l[m*128:(m+1)*128]
    bh_d = nc.declare_dram_parameter("bh", [128, 6], F32, isOutput=False)
    # Output-layer bias pre-scaled [64, 5]: cols = 0.5*b3, 0.75*b3, c1*b3, c2*b3, c3*b3
    bv_d = nc.declare_dram_parameter("bv", [D, 5], F32, isOutput=False)
    ys_d = nc.declare_dram_parameter("ysT", [T, D, BC], F32, isOutput=True)

    with tile.TileContext(nc) as tc:
        with (
            tc.tile_pool(name="const", bufs=1) as cpool,
            tc.tile_pool(name="gb", bufs=3) as gbpool,
            tc.tile_pool(name="y", bufs=4) as ypool,
            tc.tile_pool(name="xp", bufs=6) as xpool,
            tc.tile_pool(name="u", bufs=4) as upool,
            tc.tile_pool(name="h", bufs=12) as hpool,
            tc.tile_pool(name="l0ps", bufs=2, space="PSUM") as l0pool,
            tc.tile_pool(name="ps", bufs=4, space="PSUM") as pspool,
            tc.tile_pool(name="kps", bufs=2, space="PSUM") as kpool,
        ):
            # --- load constants/weights to SBUF ---
            wt0 = cpool.tile([D, W], F16, tag="wt0")
            nc.sync.dma_start(wt0[:], w0T_d[:])
            wt1 = [cpool.tile([128, W], F16, tag=f"wt1_{k}", name=f"wt1_{k}") for k in range(2)]
            wt2 = [cpool.tile([128, W], F16, tag=f"wt2_{k}", name=f"wt2_{k}") for k in range(2)]
            wt3 = [cpool.tile([128, D], F16, tag=f"wt3_{k}", name=f"wt3_{k}") for k in range(2)]
            for k in range(2):
                nc.sync.dma_start(wt1[k][:], w1T_d[k * 128:(k + 1) * 128, :])
                nc.sync.dma_start(wt2[k][:], w2T_d[k * 128:(k + 1) * 128, :])
                nc.sync.dma_start(wt3[k][:], w3T_d[k * 128:(k + 1) * 128, :])
            bh = cpool.tile([128, 6], F32, tag="bh")
            nc.sync.dma_start(bh[:], bh_d[:])
            bv = cpool.tile([D, 5], F32, tag="bv")
            nc.sync.dma_start(bv[:], bv_d[:])

            y0 = ypool.tile([D, BC], F32)
            nc.sync.dma_start(y0[:], y0T_d[:])

            y_r0 = xpool.tile([D, BC], F16, tag="x")
            for S in STREAMS:
                nc.vector.tensor_copy(y_r0[:, S], y0[:, S])

            # mlp tiles per eval (ps0/h shared between streams; kp per stream)
            ev_tiles: dict = {}
            xs = {0: y_r0}        # L0 input tile per eval
            ys_state = {0: y0}    # carried f32 state per step
            y_news: dict = {}     # y_new tile per step
            us: dict = {}         # u tile per (step, phase)

            def mlp_layer(s, S, ev, li):
                """Emit one stream's ops for one layer (0..2) or L3 (li==3).
                Per-layer emission lets the main loop interleave A and B so
                the in-order PE queue matches dependency-fire order (A leads
                B by the gate lag) and neither stream head-of-line blocks
                the other. Returns kp for li==3."""
                if s == 0 and li == 0:
                    ev_tiles[ev] = {
                        "ps0": [l0pool.tile([128, BC], F32, tag="l0", name="l0")
                                for _ in range(2)],
                        "h": [[hpool.tile([128, BC], F16, tag="h", name="h")
                               for _ in range(2)] for _ in range(3)],
                        "ps": {},
                    }
                    ev_tiles.pop(ev - 2, None)
                tl = ev_tiles[ev]
                hs = tl["h"]
                if li == 3:
                    h = hs[2]
                    kp = kpool.tile([D, H], F32, tag="kp", name="kp")
                    for k in range(2):
                        nc.tensor.matmul(kp[:], wt3[k][:], h[k][:, S],
                                         start=(k == 0), stop=(k == 1))
                    return kp
                if li == 0:
                    x = xs[ev]
                    ps0 = tl["ps0"]
                    for m in range(2):
                        nc.tensor.matmul(ps0[m][:, S],
                                         wt0[:, m * 128:(m + 1) * 128],
                                         x[:, S], start=True, stop=True)
                    for m in range(2):
                        nc.scalar.activation(ps0[m][:, S], ps0[m][:, S], AF.Exp,
                                             bias=bh[:, m:m + 1])
                        nc.scalar.activation(hs[0][m][:, S], ps0[m][:, S],
                                             AF.Ln, bias=1.0)
                    return None
                wts, bcol = ((wt1, 2), (wt2, 4))[li - 1]
                h = hs[li - 1]
                ps = [pspool.tile([128, H], F32, tag="ps", name="ps")
                      for _ in range(2)]
                for m in range(2):
                    ms = slice(m * 128, (m + 1) * 128)
                    nc.tensor.matmul(ps[m][:], wts[0][:, ms], h[0][:, S],
                                     start=True, stop=False)
                for m in range(2):
                    ms = slice(m * 128, (m + 1) * 128)
                    nc.tensor.matmul(ps[m][:], wts[1][:, ms], h[1][:, S],
                                     start=False, stop=True)
                h2 = hs[li]
                for m in range(2):
                    nc.scalar.activation(ps[m][:], ps[m][:], AF.Exp,
                                         bias=bh[:, bcol + m:bcol + m + 1])
                    nc.scalar.activation(h2[m][:, S], ps[m][:],
                                         AF.Ln, bias=1.0)
                return None

            def affines_stream(s, S, ev, kp, gate):
                """RK affines for stream s after eval ev: x_{ev+1} and u.
                gate: optional [64,1] AP used as the x-affine bias (data-dep
                pin against the other stream's progress)."""
                t, ph = divmod(ev, 3)
                y = ys_state[t]
                xscale, xcol = PHASE_XSCALE[ph], PHASE_XCOL[ph]
                uscale, ucol = PHASE_USCALE[ph], PHASE_UCOL[ph]
                if s == 0:
                    xs[ev + 1] = xpool.tile([D, BC], F16, tag="x", name="x")
                    xs.pop(ev - 2, None)
                xt = xs[ev + 1]
                xbias = gate if gate is not None else bv[:, xcol:xcol + 1]
                if ph == 2:
                    base = us[(t, 1)]
                    nc.vector.affine_then_add(xt[:, S], kp[:], base[:, S],
                                              xscale, xbias)
                    if s == 0:
                        y_news[t] = ypool.tile([D, BC], F32, name="yn")
                        ys_state[t + 1] = y_news[t]
                    yn = y_news[t]
                    nc.vector.affine_then_add(yn[:, S], kp[:], base[:, S],
                                              uscale, bv[:, ucol:ucol + 1])
                    if s == 1:
                        nc.sync.dma_start(ys_d[t, :, :], yn[:])
                        us.pop((t, 0), None)
                        us.pop((t, 1), None)
                        ys_state.pop(t - 1, None)
                        y_news.pop(t - 1, None)
                else:
                    base = y if ph == 0 else us[(t, 0)]
                    nc.vector.affine_then_add(xt[:, S], kp[:], base[:, S],
                                              xscale, xbias)
                    if s == 0:
                        us[(t, ph)] = upool.tile([D, BC], F32, tag="u", name="u")
                    ut = us[(t, ph)]
                    nc.vector.affine_then_add(ut[:, S], kp[:], base[:, S],
                                              uscale, bv[:, ucol:ucol + 1])

            def make_gate(ev, ph_prev):
                """[64,1] copy of the x-affine bias column, data-dependent on
                stream A's ln(L0, m0) of eval ev: (h*0 + 0) + bv = bv.
                ev==1 reads an h tile that is freshly pool-allocated and not
                yet written (uninitialized SBUF can hold NaN bit patterns;
                NaN*0 = NaN poisons stream B) — gate on the always-loaded bh
                constant instead for that first gated eval."""
                col = PHASE_XCOL[ph_prev]
                gb = gbpool.tile([D, 1], F32, tag="gb", name="gb")
                # h[0][0] = A's L0-m0 ln output of eval ev, already emitted in
                # program order -> a true RAW dep (h[1][0] was a read-before-
                # write of a possibly-uninitialized/racing buffer).
                src = ev_tiles[ev]["h"][0][0]
                nc.vector.affine_then_add(gb[:], src[0:D, 0:1],
                                          bv[:, col:col + 1], 0.0, 0.0)
                return gb[:, 0:1]

            # Per-layer interleave: A leads; B's layer li emitted after A's
            # layer li+1 so the PE queue order matches dep-fire order.
            A, Bs = STREAMS
            kp_B_prev = None
            for ev in range(NEV):
                mlp_layer(0, A, ev, 0)
                if ev > 0:
                    gate = make_gate(ev, (ev - 1) % 3)
                    affines_stream(1, Bs, ev - 1, kp_B_prev, gate)
                mlp_layer(0, A, ev, 1)
                mlp_layer(1, Bs, ev, 0)
                mlp_layer(0, A, ev, 2)
                mlp_layer(1, Bs, ev, 1)
                kp_A = mlp_layer(0, A, ev, 3)
                mlp_layer(1, Bs, ev, 2)
                affines_stream(0, A, ev, kp_A, None)
                kp_B_prev = mlp_layer(1, Bs, ev, 3)
            affines_stream(1, Bs, NEV - 1, kp_B_prev, None)

    nc.finalize()
    return nc


_PROGRAM_CACHE: dict = {}


def _get_program(dt: np.float32) -> bass.Bass:
    key = float(dt)
    if key not in _PROGRAM_CACHE:
        _PROGRAM_CACHE[key] = _build_program(dt)
    return _PROGRAM_CACHE[key]


def _make_in_maps(ts, y0, w0, b0, w1, b1, w2, b2, w3, b3):
    f = np.float32
    dt = f(np.asarray(ts, np.float32)[1] - np.asarray(ts, np.float32)[0])
    c1 = f(f(2.0 / 9.0) * dt)
    c2 = f(f(1.0 / 3.0) * dt)
    c3 = f(f(4.0 / 9.0) * dt)

    def _h(a):  # host-side cast to fp16 (RNE)
        return np.ascontiguousarray(np.asarray(a, np.float32).astype(np.float16))

    w0T = _h(np.asarray(w0, np.float32).T)  # [64, 256]
    w1T = _h(np.asarray(w1, np.float32).T)  # [256, 256]
    w2T = _h(np.asarray(w2, np.float32).T)
    w3T = _h(np.asarray(w3, np.float32).T)  # [256, 64]

    bh = np.empty((128, 6), np.float32)
    for li, b in enumerate((b0, b1, b2)):
        b = np.asarray(b, np.float32)
        bh[:, 2 * li] = b[0:128]
        bh[:, 2 * li + 1] = b[128:256]

    b3 = np.asarray(b3, np.float32)
    bv = np.stack([f(0.5) * b3, f(0.75) * b3, c1 * b3, c2 * b3, c3 * b3],
                  axis=1).astype(np.float32)  # [64, 5]

    y0 = np.asarray(y0, np.float32)
    in_maps = []
    for i in range(N_CORES):
        y0T = np.ascontiguousarray(y0[i * BC:(i + 1) * BC, :].T)  # [64, 512]
        in_maps.append({
            "y0T": y0T, "w0T": w0T, "w1T": w1T, "w2T": w2T, "w3T": w3T,
            "bh": bh, "bv": bv,
        })
    return dt, in_maps


def _assemble(results) -> np.ndarray:
    out = np.empty((T, B, D), np.float32)
    for i in range(N_CORES):
        ysT = results[i]["ysT"]  # [T, 64, 512]
        out[:, i * BC:(i + 1) * BC, :] = ysT.transpose(0, 2, 1)
    return out


def kernel(ts, y0, w0, b0, w1, b1, w2, b2, w3, b3) -> np.ndarray:
    dt, in_maps = _make_in_maps(ts, y0, w0, b0, w1, b1, w2, b2, w3, b3)
    nc = _get_program(dt)
    res = run_bass_kernel_spmd(nc, in_maps, list(range(N_CORES)))
    return _assemble(res.results)


def kernel_profiled(ts, y0, w0, b0, w1, b1, w2, b2, w3, b3):
    """Like kernel() but with NTFF tracing; returns (out, exec_time_ns)."""
    dt, in_maps = _make_in_maps(ts, y0, w0, b0, w1, b1, w2, b2, w3, b3)
    nc = _get_program(dt)
    res = run_bass_kernel_spmd(nc, in_maps, list(range(N_CORES)), trace=True)
    return _assemble(res.results), res

